# revision 26
# baseline (speedup 1.0000x reference)
"""Trainium2 Bass kernel for nn_CrossAttention (B=8, L=2048, DA=DB=1024, H=512).

Data-parallel over batch across 8 NeuronCores (1 batch element per core).

Math per core (inputs A, B [L, D]; Wa, Wb [D, H]; ba, bb [H]):
  ma = A@Wa + ba ; mb = B@Wb + bb       (projections)
  s  = ma @ mb^T                        [L, L]
  E  = exp(s - MHAT)                    MHAT is a static shift: both softmaxes
                                        are invariant to a global offset, so
                                        row/col sums normalize exactly.
  out_b = E^T @ (A / rowsum(E))         (row softmax folded into rhs scale)
  out_a = (E^T @ B) / colsum(E)         (col softmax via per-row post-scale)

Engine placement: PE does transposes (f32r 1.5 cy/row via bitcast, no convert
copies), projections + scores (f32r / fp16 at 1 cy/row), and the two output
matmuls (bf16). Column sums run on the otherwise-idle Pool/GpSimd engine
(cross-partition tensor_reduce); row sums ride the Exp activation's accum_out.
X tiles (xa/xb) are produced during the score phase so the output matmuls
start without stalls. No collectives; host shards/gathers.
"""

import sys

for _p in ("/opt/trn_rl_repo", "/root/.axon_site/_ro/trn_rl_repo"):
    if _p not in sys.path:
        sys.path.insert(0, _p)

import numpy as np

import concourse.bacc as bacc
import concourse.mybir as mybir
import concourse.tile as tile
from concourse.bass_utils import run_bass_kernel_spmd
from concourse.masks import make_identity

dt = mybir.dt
AF = mybir.ActivationFunctionType
AX = mybir.AxisListType
ALU = mybir.AluOpType

L, D, H = 2048, 1024, 512
NCORES = 8
LC = L // 128   # 16 row chunks
KC = D // 128   # 8 contraction chunks (projections)
HC = H // 128   # 4 H chunks
LS = L // 512   # 4 column spans of the L axis
DS = D // 512   # 2 column spans of the D axis
MHAT = 100.0    # static softmax shift (logits ~N(0, 512); global max << 188)
FP16_FRONT = True   # fp16 A^T/W/transposes (1 cy/row) vs f32r (2 cy/row T)

_CACHE = {}


def _build():
    nc = bacc.Bacc("TRN2", target_bir_lowering=False, debug=False, num_devices=NCORES)
    a_d = nc.dram_tensor("input_a", [L, D], dt.float32, kind="ExternalInput").ap()
    b_d = nc.dram_tensor("input_b", [L, D], dt.float32, kind="ExternalInput").ap()
    wa_d = nc.dram_tensor("Wa", [D, H], dt.float32, kind="ExternalInput").ap()
    ba_d = nc.dram_tensor("ba", [H], dt.float32, kind="ExternalInput").ap()
    wb_d = nc.dram_tensor("Wb", [D, H], dt.float32, kind="ExternalInput").ap()
    bb_d = nc.dram_tensor("bb", [H], dt.float32, kind="ExternalInput").ap()
    oa_d = nc.dram_tensor("out_a", [L, D], dt.float32, kind="ExternalOutput").ap()
    ob_d = nc.dram_tensor("out_b", [L, D], dt.float32, kind="ExternalOutput").ap()

    with tile.TileContext(nc) as tc:
        _body(tc, nc, a_d, b_d, wa_d, ba_d, wb_d, bb_d, oa_d, ob_d)
    nc.compile()
    return nc


def _body(tc, nc, a_d, b_d, wa_d, ba_d, wb_d, bb_d, oa_d, ob_d):
    f32, f32r, f16, bf16 = dt.float32, dt.float32r, dt.float16, dt.bfloat16

    with tc.tile_pool(name="cst", bufs=1) as cst, \
         tc.tile_pool(name="stats", bufs=1) as stp, \
         tc.tile_pool(name="big", bufs=1) as big:

        id32 = cst.tile([128, 128], f32, tag="id")
        one11 = cst.tile([1, 1], f32, tag="one11")
        onec = cst.tile([128, 1], bf16, tag="onec")
        negm = cst.tile([128, 1], f32, tag="negm")
        ba_t = cst.tile([128, HC], f32, tag="ba")
        bb_t = cst.tile([128, HC], f32, tag="bb")
        make_identity(nc, id32[:])
        nc.gpsimd.memset(one11[:], 1.0)
        nc.gpsimd.memset(onec[:], 1.0)
        nc.gpsimd.memset(negm[:], -MHAT)
        nc.scalar.dma_start(ba_t[:], ba_d.rearrange("(c p) -> p c", p=128))
        nc.scalar.dma_start(bb_t[:], bb_d.rearrange("(c p) -> p c", p=128))

        # mapped projections, transposed: maT[p, hc, i] = ma[i, hc*128+p]
        maT = big.tile([128, HC, L], f16, tag="maT")
        mbT = big.tile([128, HC, L], f16, tag="mbT")

        rsp = stp.tile([128, LC * LS], f32, tag="rsp")    # per-span exp sums
        rs1 = stp.tile([128, LC], f32, tag="rs1")         # row sums
        rrs = stp.tile([128, LC], f32, tag="rrs")         # 1/rowsum
        crow = stp.tile([1, L], f32, tag="crow")          # colsum -> 1/colsum
        rcs = stp.tile([128, LC], f32, tag="rcs")         # 1/colsum, relaid

        # ---------------- Phase 1: transposes + projections ------------------
        tdt = f16 if FP16_FRONT else f32r
        with tc.tile_pool(name="wp", bufs=1) as wp, \
             tc.tile_pool(name="wsp", bufs=2) as wsp, \
             tc.tile_pool(name="natp", bufs=1) as natp, \
             tc.tile_pool(name="n16p", bufs=4) as n16p, \
             tc.tile_pool(name="aTp", bufs=1) as atp, \
             tc.tile_pool(name="psT", bufs=2, space="PSUM") as psT, \
             tc.tile_pool(name="psP", bufs=4, space="PSUM") as psP:

            wa_t = wp.tile([128, KC, H], tdt, tag="wa")
            wb_t = wp.tile([128, KC, H], tdt, tag="wb")
            idT = wp.tile([128, 128], tdt, tag="idT")
            nc.gpsimd.tensor_copy(idT[:], id32[:])
            aT = atp.tile([128, KC, L], tdt, tag="aT")   # shared slot for A, B

            for src_d, w_d, w_t, bias_t, mT, ntag, nbufs in (
                    (a_d, wa_d, wa_t, ba_t, maT, "na", 6),
                    (b_d, wb_d, wb_t, bb_t, mbT, "nb", 4)):
                # DMA weave: first nat tiles, then W h-blocks, then the rest.
                # W-staging copies run on the idle Pool engine so the DVE
                # stays free for the aT evacuations.
                nat = {}
                for ic in range(4):
                    nat[ic] = natp.tile([128, D], f32, tag=ntag, bufs=nbufs,
                                        name=f"{ntag}{ic}")
                    nc.sync.dma_start(nat[ic][:],
                                      src_d[ic * 128:(ic + 1) * 128, :])
                for hc in range(HC):
                    # W rides the scalar DMA queue, in parallel with the
                    # nat-tile stream on the sync queue
                    wst = wsp.tile([128, KC, 128], f32, tag="wst")
                    nc.scalar.dma_start(
                        wst[:], w_d[:, hc * 128:(hc + 1) * 128].rearrange(
                            "(c p) h -> p c h", p=128))
                    nc.gpsimd.tensor_copy(
                        w_t[:, :, hc * 128:(hc + 1) * 128], wst[:])
                for ic in range(4, LC):
                    nat[ic] = natp.tile([128, D], f32, tag=ntag, bufs=nbufs,
                                        name=f"{ntag}{ic}")
                    nc.sync.dma_start(nat[ic][:],
                                      src_d[ic * 128:(ic + 1) * 128, :])

                def tgroup(S):
                    # transpose the 4 row chunks of span S into aT
                    for ic in range(4 * S, 4 * S + 4):
                        if FP16_FRONT:
                            n16 = n16p.tile([128, D], f16, tag="n16")
                            nc.scalar.copy(n16[:], nat[ic][:])
                            tsrc = n16
                        else:
                            tsrc = nat[ic]
                        pt = psT.tile([128, D], f16 if FP16_FRONT else f32,
                                      tag="pt")
                        for dc in range(KC):
                            nc.tensor.transpose(
                                pt[:, dc * 128:(dc + 1) * 128],
                                tsrc[:, dc * 128:(dc + 1) * 128],
                                idT[:] if FP16_FRONT else id32[:])
                        nc.vector.tensor_copy(
                            aT[:, :, ic * 128:(ic + 1) * 128],
                            pt.rearrange("p (c i) -> p c i", c=KC))

                # delay-by-one: proj(S) is emitted after tgroup(S+1), so the
                # PE never waits on the aT evacuations of its own span.
                tgroup(0)
                for S in range(LS):
                    if S + 1 < LS:
                        tgroup(S + 1)
                    for hc in range(HC):
                        pp = psP.tile([128, 512], f32, tag="pp")
                        for dc in range(KC):
                            nc.tensor.matmul(
                                pp[:],
                                w_t[:, dc, hc * 128:(hc + 1) * 128],
                                aT[:, dc, S * 512:(S + 1) * 512],
                                start=(dc == 0), stop=(dc == KC - 1))
                        nc.scalar.activation(
                            mT[:, hc, S * 512:(S + 1) * 512], pp[:],
                            AF.Identity, bias=bias_t[:, hc:hc + 1])

        # ---------------- Phase 2: scores, E, sums, X tiles -------------------
        with tc.tile_pool(name="big2", bufs=1) as big2:
            E = big2.tile([128, LC, L], bf16, tag="E")
            xa = big2.tile([128, LC, D], bf16, tag="xa")
            xb = big2.tile([128, LC, D], bf16, tag="xb")

            with tc.tile_pool(name="psS", bufs=4, space="PSUM") as psS, \
                 tc.tile_pool(name="psC", bufs=1, space="PSUM") as psC, \
                 tc.tile_pool(name="natx", bufs=1) as nxp:
                pcs = [psC.tile([1, 512], f32, tag=f"pcs{q}", name=f"pcs{q}")
                       for q in range(LS)]

                def colsum(i):
                    # ones-matmul partial column sums, PSUM-accumulated
                    for q in range(LS):
                        nc.tensor.matmul(
                            pcs[q][:], onec[:],
                            E[:, i, q * 512:(q + 1) * 512],
                            start=(i == 0), stop=(i == LC - 1))

                for i in range(LC):
                    isl = slice(i * 128, (i + 1) * 128)
                    for q in range(LS):
                        ps = psS.tile([128, 512], f32, tag="ps")
                        for hc in range(HC):
                            nc.tensor.matmul(
                                ps[:], maT[:, hc, isl],
                                mbT[:, hc, q * 512:(q + 1) * 512],
                                start=(hc == 0), stop=(hc == HC - 1))
                        nc.scalar.activation(
                            E[:, i, q * 512:(q + 1) * 512], ps[:], AF.Exp,
                            bias=negm[:],
                            accum_out=rsp[:, i * LS + q:i * LS + q + 1])
                    if i > 0:
                        colsum(i - 1)   # one chunk late: exp(i-1) is done
                    # row sums + reciprocal for this chunk
                    nc.vector.tensor_reduce(rs1[:, i:i + 1],
                                            rsp[:, i * LS:(i + 1) * LS],
                                            axis=AX.X, op=ALU.add)
                    nc.vector.reciprocal(rrs[:, i:i + 1], rs1[:, i:i + 1])
                    # X tiles for the output matmuls
                    na = nxp.tile([128, D], f32, tag="nxa", bufs=2)
                    nc.sync.dma_start(na[:], a_d[isl, :])
                    nc.vector.tensor_scalar_mul(xa[:, i, :], na[:],
                                                rrs[:, i:i + 1])
                    nb = nxp.tile([128, D], f32, tag="nxb", bufs=2)
                    nc.sync.dma_start(nb[:], b_d[isl, :])
                    nc.scalar.copy(xb[:, i, :], nb[:])
                colsum(LC - 1)

                # total column sums -> reciprocal (evacs split across DVE
                # and ACT so the PSUM banks free up faster for phase 5)
                for q in range(LS):
                    dst = crow[0:1, q * 512:(q + 1) * 512]
                    if q % 2 == 0:
                        nc.vector.tensor_copy(dst, pcs[q][:])
                    else:
                        nc.scalar.copy(dst, pcs[q][:])
                nc.vector.reciprocal(crow[:], crow[:])

            # ---------------- Phase 5: output matmuls ------------------------
            with tc.tile_pool(name="psR", bufs=1, space="PSUM") as psRp, \
                 tc.tile_pool(name="pmm5", bufs=7, space="PSUM") as pmm, \
                 tc.tile_pool(name="outp", bufs=4) as outp:

                def relayout():
                    # 1/colsum row -> [128, LC] per-chunk columns via PE
                    psR = psRp.tile([128, LC], f32, tag="psr")
                    for c in range(LC):
                        nc.tensor.matmul(psR[:, c:c + 1],
                                         crow[0:1, c * 128:(c + 1) * 128],
                                         one11[:], start=True, stop=True)
                    nc.vector.tensor_copy(rcs[:], psR[:])

                def osa_drain(poa, cc, csl, dsl):
                    osa = outp.tile([128, 512], f32, tag="oa")
                    nc.vector.tensor_scalar_mul(osa[:], poa[:],
                                                rcs[:, cc:cc + 1])
                    nc.scalar.dma_start(oa_d[csl, dsl], osa[:])

                niter = 0
                pending = []
                for ds in range(DS):
                    dsl = slice(ds * 512, (ds + 1) * 512)
                    for c in range(LC):
                        csl = slice(c * 128, (c + 1) * 128)
                        last = (ds == DS - 1 and c == LC - 1)
                        pob = pmm.tile([128, 512], f32, tag="mm")
                        poa = pmm.tile([128, 512], f32, tag="mm")
                        if last:
                            # de-interleave + split the final out_a chain in
                            # two half-width chains so every drain overlaps
                            # remaining PE work at the kernel tail
                            for k in range(LC):
                                nc.tensor.matmul(pob[:], E[:, k, csl],
                                                 xa[:, k, dsl], start=(k == 0),
                                                 stop=(k == LC - 1))
                            osb = outp.tile([128, 512], f32, tag="ob")
                            nc.scalar.copy(osb[:], pob[:])
                            nc.sync.dma_start(ob_d[csl, dsl], osb[:])
                            for k in range(LC):
                                nc.tensor.matmul(poa[:, 0:256], E[:, k, csl],
                                                 xb[:, k, ds * 512:ds * 512 + 256],
                                                 start=(k == 0),
                                                 stop=(k == LC - 1))
                            osa1 = outp.tile([128, 256], f32, tag="oa1")
                            nc.vector.tensor_scalar_mul(osa1[:], poa[:, 0:256],
                                                        rcs[:, c:c + 1])
                            nc.scalar.dma_start(
                                oa_d[csl, ds * 512:ds * 512 + 256], osa1[:])
                            for k in range(LC):
                                nc.tensor.matmul(
                                    poa[:, 256:512], E[:, k, csl],
                                    xb[:, k, ds * 512 + 256:(ds + 1) * 512],
                                    start=(k == 0), stop=(k == LC - 1))
                            osa2 = outp.tile([128, 256], f32, tag="oa2")
                            nc.vector.tensor_scalar_mul(osa2[:], poa[:, 256:512],
                                                        rcs[:, c:c + 1])
                            nc.scalar.dma_start(
                                oa_d[csl, ds * 512 + 256:(ds + 1) * 512],
                                osa2[:])
                            continue
                        else:
                            for k in range(LC):
                                esl = E[:, k, csl]
                                nc.tensor.matmul(pob[:], esl, xa[:, k, dsl],
                                                 start=(k == 0),
                                                 stop=(k == LC - 1))
                                nc.tensor.matmul(poa[:], esl, xb[:, k, dsl],
                                                 start=(k == 0),
                                                 stop=(k == LC - 1))
                            osb = outp.tile([128, 512], f32, tag="ob")
                            nc.scalar.copy(osb[:], pob[:])
                            nc.sync.dma_start(ob_d[csl, dsl], osb[:])
                        if niter < 2:
                            # defer: rcs is not written until relayout()
                            pending.append((poa, c, csl, dsl))
                        else:
                            osa_drain(poa, c, csl, dsl)
                        niter += 1
                        if niter == 2:
                            # emit the colsum relayout only now: its inputs
                            # arrive ~5us after phase 2 ends, and nothing on
                            # the PE needs it (only the DVE post-scale does)
                            relayout()
                            for args in pending:
                                osa_drain(*args)
                            pending = []


def _execute(inputs, trace=False):
    if "nc" not in _CACHE:
        _CACHE["nc"] = _build()
    nc = _CACHE["nc"]

    f32 = np.float32
    Wa = np.ascontiguousarray(np.asarray(inputs["Wa"], dtype=f32))
    Wb = np.ascontiguousarray(np.asarray(inputs["Wb"], dtype=f32))
    ba = np.ascontiguousarray(np.asarray(inputs["ba"], dtype=f32))
    bb = np.ascontiguousarray(np.asarray(inputs["bb"], dtype=f32))
    ia = np.asarray(inputs["input_a"], dtype=f32)
    ib = np.asarray(inputs["input_b"], dtype=f32)

    in_maps = []
    for c in range(NCORES):
        in_maps.append({
            "input_a": np.ascontiguousarray(ia[c]),
            "input_b": np.ascontiguousarray(ib[c]),
            "Wa": Wa, "ba": ba, "Wb": Wb, "bb": bb,
        })
    res = run_bass_kernel_spmd(nc, in_maps, list(range(NCORES)), trace=trace)
    out_a = np.stack([res.results[c]["out_a"] for c in range(NCORES)])
    out_b = np.stack([res.results[c]["out_b"] for c in range(NCORES)])
    return (out_a, out_b), res


def kernel(**inputs):
    (out_a, out_b), _ = _execute(inputs, trace=False)
    return (out_a, out_b)
